# revision 6
# baseline (speedup 1.0000x reference)
"""Trainium2 Bass kernel for a pre-norm transformer block (attention + MLP).

Sharding: batch (2) x query-block (4) across 8 cores. Each core computes
LN1 + K/V over its full batch (replicated within its 4-core group) and
attention / projection / MLP for its own 1024 query tokens. No collectives.

v2: fp8 DoubleRow matmuls for QKV/S/wp (2x PE throughput), exp->bf16 on the
Act engine (the measured bottleneck: 0.733ns/elem), LN rstd via Ln+Exp
(keeps the single natural_log_exp activation table loaded), LN normalize on
DVE, attention emission software-pipelined so Act stays saturated.

Device layouts (per core):
  xTb : LN1(x) feature-major [128ci, 32mt, 4kt, 128t] bf16 (DMA transpose)
  xT8 : [128ci, 4kt, 32mt, 128t] fp8 (Pool convert; kt-major for DR pairs)
  kT  : per head-group [128 (4h x 32 d_lo), 2 d_hi, 4096m] fp8
  qT  : per head-group [128, 2, 1024n] fp8
  v   : token-major [128m, 32mt, 8h, 65] bf16 (65th col = softmax denom ones)
  S^T : psum [128m, 2mt, 512n] f32 per (head, nch, mt-pair); exp -> pt bf16
  A@V : po[65, 512] psum accumulated over 32 mt (lhsT = [V|1])
  ao  : [64d, 8h, 1024n] fp8 -> wp via DoubleRow over head pairs
"""

import numpy as np
import ml_dtypes

B, N, C = 2, 4096, 512
H, D = 8, 64
HID = 2048
NQ = 1024
NCORES = 8
EPS = 1e-5
BF = ml_dtypes.bfloat16
E4 = ml_dtypes.float8_e4m3

_CACHE = {}


def _build_program(repeat=1):
    from concourse import bacc
    import concourse.bass as bass
    import concourse.mybir as mybir
    from concourse.tile import TileContext

    dt = mybir.dt
    AF = mybir.ActivationFunctionType
    ALU = mybir.AluOpType
    DR = mybir.MatmulPerfMode.DoubleRow

    nc = bacc.Bacc(None, target_bir_lowering=False)

    xfull = nc.dram_tensor("xfull", (N, C), dt.float32, kind="ExternalInput")
    xq = nc.dram_tensor("xq", (NQ, C), dt.float32, kind="ExternalInput")
    wq_d = nc.dram_tensor("wq_d", (128, 4, C), dt.float8e4, kind="ExternalInput")
    wk_d = nc.dram_tensor("wk_d", (128, 4, C), dt.float8e4, kind="ExternalInput")
    wv_d = nc.dram_tensor("wv_d", (128, 4, C), dt.float8e4, kind="ExternalInput")
    wp_d = nc.dram_tensor("wp_d", (64, 8, C), dt.float8e4, kind="ExternalInput")
    w1_d = nc.dram_tensor("w1_d", (128, 4, HID), dt.bfloat16, kind="ExternalInput")
    w2_d = nc.dram_tensor("w2_d", (128, 16, C), dt.bfloat16, kind="ExternalInput")
    bq_d = nc.dram_tensor("bq_d", (128, 4), dt.float32, kind="ExternalInput")
    bk_d = nc.dram_tensor("bk_d", (128, 4), dt.float32, kind="ExternalInput")
    bv_d = nc.dram_tensor("bv_d", (C,), dt.float32, kind="ExternalInput")
    bp_d = nc.dram_tensor("bp_d", (C,), dt.float32, kind="ExternalInput")
    b1_d = nc.dram_tensor("b1_d", (128, 16), dt.float32, kind="ExternalInput")
    b2_d = nc.dram_tensor("b2_d", (C,), dt.float32, kind="ExternalInput")
    y = nc.dram_tensor("y", (NQ, C), dt.float32, kind="ExternalOutput")

    xq_t = xq.rearrange("(i p) c -> p i c", p=128)
    y_t = y.rearrange("(i p) c -> p i c", p=128)

    import contextlib
    with TileContext(nc) as tc:
      with (tc.For_i(0, repeat, 1) if repeat > 1 else contextlib.nullcontext()):
        R = "r0_"
        with tc.tile_pool(name=R + "pers", bufs=1) as pers, \
             tc.tile_pool(name=R + "stat", bufs=4) as statp, \
             tc.tile_pool(name=R + "stream", bufs=3) as stream, \
             tc.tile_pool(name=R + "pB", bufs=1) as pB, \
             tc.tile_pool(name=R + "ptp", bufs=4) as ptp, \
             tc.tile_pool(name=R + "pall", bufs=1, space="PSUM") as pall:

            eps_t = pers.tile([128, 1], dt.float32, name=R + "eps")
            nc.vector.memset(eps_t, EPS)
            ones_sb = pers.tile([128, 64], dt.bfloat16, name=R + "ones")
            nc.vector.memset(ones_sb, 1.0)
            xq_sb = pers.tile([128, 8, C], dt.float32, name=R + "xq_sb")
            nc.sync.dma_start(out=xq_sb, in_=xq_t[:])
            bq_sb = pers.tile([128, 4], dt.float32, name=R + "bq_sb")
            bk_sb = pers.tile([128, 4], dt.float32, name=R + "bk_sb")
            bv_sb = pers.tile([128, 8, 64], dt.bfloat16, name=R + "bv_sb")
            bp_sb = pers.tile([128, C], dt.float32, name=R + "bp_sb")
            b1_sb = pers.tile([128, 16], dt.float32, name=R + "b1_sb")
            b2_sb = pers.tile([128, C], dt.float32, name=R + "b2_sb")
            nc.sync.dma_start(out=bq_sb, in_=bq_d[:])
            nc.sync.dma_start(out=bk_sb, in_=bk_d[:])
            nc.sync.dma_start(out=b1_sb, in_=b1_d[:])
            nc.gpsimd.dma_start(out=bv_sb, in_=bass.AP(tensor=bv_d, offset=0, ap=[[0, 128], [1, C]]))
            nc.gpsimd.dma_start(out=bp_sb, in_=bass.AP(tensor=bp_d, offset=0, ap=[[0, 128], [1, C]]))
            nc.gpsimd.dma_start(out=b2_sb, in_=bass.AP(tensor=b2_d, offset=0, ap=[[0, 128], [1, C]]))

            # persistent attention tiles
            wp_sb = pB.tile([64, 8, C], dt.float8e4, name=R + "wp_sb")
            nc.gpsimd.dma_start(out=wp_sb, in_=wp_d[:])
            v_sb = pB.tile([128, 32, H, 65], dt.bfloat16, name=R + "v_sb")
            nc.vector.memset(v_sb[:, :, :, 64:65], 1.0)
            ao_sb = pB.tile([64, 8, NQ], dt.float8e4, name=R + "ao")
            kT = [pB.tile([128, 2, N], dt.float8e4, name=f"{R}kT{g}") for g in range(2)]
            qT = [pB.tile([128, 2, NQ], dt.float8e4, name=f"{R}qT{g}") for g in range(2)]

            def ln_group(src, g, xT_dst, tag, from_sbuf=False):
                """LN a group of 4 token-tiles and transpose into xT_dst.

                rstd = exp(-0.5*ln(var+eps)) for all 4 tiles in one batched
                Ln + Exp pair (keeps Act on the single ln/exp table)."""
                eng = nc.sync if g % 2 == 0 else nc.scalar
                if from_sbuf:
                    xt4 = src
                else:
                    xt4 = stream.tile([128, 4, C], dt.float32, tag="lnx", bufs=2, name=f"{R}{tag}x{g}")
                    eng.dma_start(out=xt4, in_=src)
                xn4 = stream.tile([128, 4, C], dt.bfloat16, tag="lnn", bufs=3, name=f"{R}{tag}n{g}")
                mv4 = statp.tile([128, 4, 2], dt.float32, tag="lnmv", name=f"{R}{tag}mv{g}")
                for j in range(4):
                    stats = statp.tile([128, 6], dt.float32, tag="lnst", name=f"{R}{tag}st{4*g+j}")
                    nc.vector.bn_stats(stats, xt4[:, j, :])
                    nc.vector.bn_aggr(mv4[:, j, :], stats)
                lnv = statp.tile([128, 4], dt.float32, tag="lnlv", name=f"{R}{tag}lv{g}")
                nc.scalar.activation(lnv, mv4[:, :, 1], AF.Ln, bias=eps_t)
                rstd4 = statp.tile([128, 4], dt.float32, tag="lnrs", name=f"{R}{tag}rs{g}")
                nc.scalar.activation(rstd4, lnv, AF.Exp, scale=-0.5)
                nmr4 = statp.tile([128, 4], dt.float32, tag="lnnm", name=f"{R}{tag}nm{g}")
                nc.vector.tensor_tensor(out=nmr4, in0=mv4[:, :, 0], in1=rstd4,
                                        op=ALU.mult)
                nc.vector.tensor_scalar(nmr4, nmr4, -1.0, None, ALU.mult)
                for j in range(4):
                    nc.vector.tensor_scalar(xn4[:, j, :], xt4[:, j, :],
                                            rstd4[:, j:j + 1], nmr4[:, j:j + 1],
                                            ALU.mult, ALU.add)
                eng.dma_start(out=xT_dst[:, 4 * g:4 * g + 4, :, :], in_=xn4, transpose=True)

            # ---- attention sweep machinery ----
            # sweep = (nch, hgrp, h4): 16 tp units, each: 2 S-DR matmuls,
            # 1 exp, 2 AV matmuls accumulating po.
            po_of = {}

            def sweep_unit(sw, tp):
                nch, hgrp, h4 = sw
                h = 4 * hgrp + h4
                rows = slice(32 * h4, 32 * h4 + 32)
                nsl = slice(nch * 512, (nch + 1) * 512)
                if tp == 0:
                    po_of[sw] = pall.tile([65, 512], dt.float32, bufs=2, tag="po",
                                          name=f"{R}po_{nch}_{h}")
                po = po_of[sw]
                ps_s = pall.tile([128, 2, 512], dt.float32, bufs=2, tag="ps_s",
                                 name=f"{R}ps_s{nch}_{h}_{tp}")
                for par in range(2):
                    mt = 2 * tp + par
                    nc.tensor.matmul(ps_s[:, par, :],
                                     kT[hgrp][rows, :, mt * 128:(mt + 1) * 128],
                                     qT[hgrp][rows, :, nsl],
                                     start=True, stop=True, perf_mode=DR,
                                     tile_position=(32 * h4, 0))
                pt = ptp.tile([128, 2, 512], dt.bfloat16, tag="pt",
                              name=f"{R}pt{nch}_{h}_{tp}")
                nc.scalar.activation(pt, ps_s, AF.Exp, scale=float(D) ** -0.5)
                for par in range(2):
                    nc.tensor.matmul(po, v_sb[:, 2 * tp + par, h, :], pt[:, par, :],
                                     start=(tp == 0 and par == 0),
                                     stop=(tp == 15 and par == 1))

            def sweep_finalize(sw):
                nch, hgrp, h4 = sw
                h = 4 * hgrp + h4
                nsl = slice(nch * 512, (nch + 1) * 512)
                po = po_of.pop(sw)
                rden = statp.tile([128, 512], dt.bfloat16, bufs=2, tag="rden",
                                  name=f"{R}rden{nch}_{h}")
                with nc.allow_low_precision(reason="bf16 softmax denom for cheap bcast matmul"):
                    nc.vector.reciprocal(rden[64:65, :], po[64:65, :])
                bc_ps = pall.tile([64, 512], dt.float32, bufs=2, tag="psmall",
                                  name=f"{R}bc{nch}_{h}")
                nc.tensor.matmul(bc_ps, ones_sb[64:65, :], rden[64:65, :],
                                 start=True, stop=True, tile_position=(64, 0))
                bc_sb = statp.tile([64, 512], dt.float32, bufs=2, tag="bcs",
                                   name=f"{R}bcs{nch}_{h}")
                nc.vector.tensor_copy(bc_sb, bc_ps)
                nc.vector.tensor_tensor(out=ao_sb[:, h, nsl], in0=po[0:64, :],
                                        in1=bc_sb, op=ALU.mult)

            # ======== phase A: LN1 + QKV, with the first 2 sweeps inlined ====
            with tc.tile_pool(name=R + "pA", bufs=1) as pA:
                wq_sb = pA.tile([128, 4, C], dt.float8e4, name=R + "wq_sb")
                wk_sb = pA.tile([128, 4, C], dt.float8e4, name=R + "wk_sb")
                wv_sb = pA.tile([128, 4, C], dt.float8e4, name=R + "wv_sb")
                nc.gpsimd.dma_start(out=wq_sb, in_=wq_d[:])
                nc.gpsimd.dma_start(out=wk_sb, in_=wk_d[:])
                nc.gpsimd.dma_start(out=wv_sb, in_=wv_d[:])
                xTb = pA.tile([128, 32, 4, 128], dt.bfloat16, name=R + "xTb")
                xT8 = pA.tile([128, 4, 32, 128], dt.float8e4, name=R + "xT8")
                xqTb = pA.tile([128, 8, 4, 128], dt.bfloat16, name=R + "xqTb")
                xqT8 = pA.tile([128, 4, 8, 128], dt.float8e4, name=R + "xqT8")

                def convert8(src, dst, mt0, nmt):
                    # [p, mt, kt, t] bf16 -> [p, kt, mt, t] fp8 (free-dim swap)
                    nc.gpsimd.tensor_copy(
                        dst[:, :, mt0:mt0 + nmt, :].rearrange("p k m t -> p m k t"),
                        src[:, mt0:mt0 + nmt, :, :])

                xq4_t = xq.rearrange("(gr j p) c -> p gr j c", p=128, j=4)
                for g in range(2):
                    ln_group(xq4_t[:, g, :, :], g, xqTb, "lq")
                    convert8(xqTb, xqT8, 4 * g, 4)
                # Q projections (DR over kt pairs)
                for hgrp in range(2):
                    for dhi in range(2):
                        csl = slice(hgrp * 256 + dhi * 128, hgrp * 256 + dhi * 128 + 128)
                        for nchq in range(2):
                            ps_q = pall.tile([128, 512], dt.float32, bufs=2, tag="psmall",
                                             name=f"{R}ps_q{hgrp}_{dhi}_{nchq}")
                            for kp in range(2):
                                nc.tensor.matmul(ps_q, wq_sb[:, 2 * kp:2 * kp + 2, csl],
                                                 xqT8[:, 2 * kp:2 * kp + 2,
                                                      4 * nchq:4 * nchq + 4, :],
                                                 start=(kp == 0), stop=(kp == 1),
                                                 perf_mode=DR)
                            nc.vector.tensor_scalar(
                                qT[hgrp][:, dhi, nchq * 512:(nchq + 1) * 512],
                                ps_q, bq_sb[:, 2 * hgrp + dhi:2 * hgrp + dhi + 1],
                                None, ALU.add)

                xf4_t = xfull.rearrange("(gr j p) c -> p gr j c", p=128, j=4)
                early = [(0, 0, 0), (0, 0, 1)]  # sweeps fed inside the g loop
                for g in range(8):
                    ln_group(xf4_t[:, g, :, :], g, xTb, "l1")
                    convert8(xTb, xT8, 4 * g, 4)
                    # K chunk g (both head groups, both d_hi halves)
                    for hgrp in range(2):
                        for dhi in range(2):
                            csl = slice(hgrp * 256 + dhi * 128,
                                        hgrp * 256 + dhi * 128 + 128)
                            ps_k = pall.tile([128, 512], dt.float32, bufs=2, tag="psmall",
                                             name=f"{R}ps_k{g}_{hgrp}_{dhi}")
                            for kp in range(2):
                                nc.tensor.matmul(ps_k, wk_sb[:, 2 * kp:2 * kp + 2, csl],
                                                 xT8[:, 2 * kp:2 * kp + 2,
                                                     4 * g:4 * g + 4, :],
                                                 start=(kp == 0), stop=(kp == 1),
                                                 perf_mode=DR)
                            nc.vector.tensor_scalar(
                                kT[hgrp][:, dhi, g * 512:(g + 1) * 512],
                                ps_k, bk_sb[:, 2 * hgrp + dhi:2 * hgrp + dhi + 1],
                                None, ALU.add)
                    # V chunk g
                    for mt in range(4 * g, 4 * g + 4):
                        ps_v = pall.tile([128, C], dt.float32, bufs=2, tag="psmall",
                                         name=f"{R}ps_v{mt}")
                        for kp in range(2):
                            nc.tensor.matmul(ps_v, xT8[:, 2 * kp:2 * kp + 2, mt, :],
                                             wv_sb[:, 2 * kp:2 * kp + 2, :],
                                             start=(kp == 0), stop=(kp == 1),
                                             perf_mode=DR)
                        nc.vector.tensor_tensor(
                            out=v_sb[:, mt, :, 0:64],
                            in0=ps_v.rearrange("p (h d) -> p h d", h=H),
                            in1=bv_sb, op=ALU.add)
                    # feed the first two sweeps as key chunks become ready
                    for sw in early:
                        sweep_unit(sw, 2 * g)
                        sweep_unit(sw, 2 * g + 1)

            # ======== attention sweeps + interleaved wp/LN2/MLP ========
            with tc.tile_pool(name=R + "pC", bufs=1) as pC:
                w1_sb = pC.tile([128, 4, HID], dt.bfloat16, name=R + "w1_sb")
                w2_sb = pC.tile([128, 16, C], dt.bfloat16, name=R + "w2_sb")
                nc.gpsimd.dma_start(out=w1_sb, in_=w1_d[:])
                nc.gpsimd.dma_start(out=w2_sb, in_=w2_d[:])
                x2T = pC.tile([128, 8, 4, 128], dt.bfloat16, name=R + "x2T")
                h_sb = [pC.tile([128, 16, 512], dt.bfloat16, name=f"{R}h_sb{i}")
                        for i in range(2)]

                def emit_wp(ns):
                    ps_p = pall.tile([128, C], dt.float32, bufs=2, tag="psmall",
                                     name=f"{R}ps_p{ns}")
                    qsl = slice(ns * 128, (ns + 1) * 128)
                    for j in range(4):
                        nc.tensor.matmul(ps_p, ao_sb[:, 2 * j:2 * j + 2, qsl],
                                         wp_sb[:, 2 * j:2 * j + 2, :],
                                         start=(j == 0), stop=(j == 3), perf_mode=DR)
                    nc.vector.tensor_tensor(out=xq_sb[:, ns, :], in0=xq_sb[:, ns, :],
                                            in1=ps_p, op=ALU.add)
                    nc.vector.tensor_tensor(out=xq_sb[:, ns, :], in0=xq_sb[:, ns, :],
                                            in1=bp_sb, op=ALU.add)

                def emit_ln2(g2):
                    ln_group(xq_sb.rearrange("p (gr j) c -> p gr j c", j=4)[:, g2, :, :],
                             g2, x2T, "l2", from_sbuf=True)

                def emit_fc1(nch, pt_lo, pt_hi):
                    for pt_i in range(pt_lo, pt_hi):
                        ps_h = pall.tile([128, 512], dt.float32, bufs=2, tag="psmall",
                                         name=f"{R}ps_h{pt_i}_{nch}")
                        for kt in range(4):
                            nc.tensor.matmul(ps_h, w1_sb[:, kt, pt_i * 128:(pt_i + 1) * 128],
                                             x2T[:, 4 * nch:4 * nch + 4, kt, :],
                                             start=(kt == 0), stop=(kt == 3))
                        nc.vector.tensor_scalar(h_sb[nch][:, pt_i, :],
                                                ps_h, b1_sb[:, pt_i:pt_i + 1], 0.0,
                                                ALU.add, ALU.max)

                def emit_fc2(nch, ns):
                    ps_m = pall.tile([128, C], dt.float32, bufs=2, tag="psmall",
                                     name=f"{R}ps_m{ns}")
                    qsl = slice((ns - 4 * nch) * 128, (ns - 4 * nch + 1) * 128)
                    for kt in range(16):
                        nc.tensor.matmul(ps_m, h_sb[nch][:, kt, qsl], w2_sb[:, kt, :],
                                         start=(kt == 0), stop=(kt == 15))
                    ot = stream.tile([128, C], dt.float32, tag="out", name=f"{R}out{ns}")
                    nc.vector.tensor_tensor(out=ot, in0=ps_m, in1=xq_sb[:, ns, :], op=ALU.add)
                    nc.vector.tensor_tensor(out=ot, in0=ot, in1=b2_sb, op=ALU.add)
                    nc.sync.dma_start(out=y_t[:, ns, :], in_=ot)

                sweeps = [(nch, hgrp, h4)
                          for nch in range(2) for hgrp in range(2) for h4 in range(4)]
                # nch0 tail work interleaved into the nch1 sweeps
                fillers = [
                    lambda: (emit_wp(0), emit_wp(1)),
                    lambda: (emit_wp(2), emit_wp(3)),
                    lambda: emit_ln2(0),
                    lambda: emit_fc1(0, 0, 6),
                    lambda: emit_fc1(0, 6, 11),
                    lambda: emit_fc1(0, 11, 16),
                    lambda: emit_fc2(0, 0) or emit_fc2(0, 1),
                    lambda: emit_fc2(0, 2) or emit_fc2(0, 3),
                ]
                sweep_finalize((0, 0, 0))
                sweep_finalize((0, 0, 1))
                fi = 0
                for si, sw in enumerate(sweeps):
                    if sw in ((0, 0, 0), (0, 0, 1)):
                        continue
                    for tp in range(16):
                        sweep_unit(sw, tp)
                    sweep_finalize(sw)
                    if sw[0] == 1:  # nch1 sweep done -> emit one nch0 block
                        if fi < len(fillers):
                            fillers[fi]()
                            fi += 1
                while fi < len(fillers):
                    fillers[fi]()
                    fi += 1
                # nch1 tail
                for ns in range(4, 8):
                    emit_wp(ns)
                emit_ln2(1)
                emit_fc1(1, 0, 16)
                for ns in range(4, 8):
                    emit_fc2(1, ns)

    # The act-table placement pass picks the first table containing each
    # function, ping-ponging between the Ln table and the Exp table (80+
    # LoadActFuncSet @1.28us). Ln and Exp share one table — restrict the
    # pass to it (other entries emptied, not removed, so act_func_set_id
    # indices still match act_info.json).
    import concourse.bacc as bacc_mod
    orig_tables = bacc_mod.get_activation_tables
    keep = "natural_log_exp_and_others"

    def _only_nl_exp(arch):
        return {k: (v if k == keep else set())
                for k, v in orig_tables(arch).items()}

    bacc_mod.get_activation_tables = _only_nl_exp
    try:
        nc.finalize()
    finally:
        bacc_mod.get_activation_tables = orig_tables
    return nc


def _prepare_host(inputs):
    f32 = np.float32
    x = np.asarray(inputs["x"], f32)
    ln1_w = np.asarray(inputs["ln1_w"], f32); ln1_b = np.asarray(inputs["ln1_b"], f32)
    ln2_w = np.asarray(inputs["ln2_w"], f32); ln2_b = np.asarray(inputs["ln2_b"], f32)
    wq = np.asarray(inputs["wq"], f32); wkv = np.asarray(inputs["wkv"], f32)
    wp = np.asarray(inputs["wp"], f32); bp = np.asarray(inputs["bp"], f32)
    w1 = np.asarray(inputs["w1"], f32); b1 = np.asarray(inputs["b1"], f32)
    w2 = np.asarray(inputs["w2"], f32); b2 = np.asarray(inputs["b2"], f32)

    wq_f = ln1_w[:, None] * wq
    wkv_f = ln1_w[:, None] * wkv
    w1_f = ln2_w[:, None] * w1
    bq_f = ln1_b @ wq
    bkv_f = ln1_b @ wkv
    b1_f = b1 + ln2_b @ w1

    # feature permutation for the 32-partition DoubleRow S layout:
    # new position = hgrp*256 + d_hi*128 + (h%4)*32 + d_lo
    perm = np.empty(C, np.int64)
    for h in range(H):
        for d in range(D):
            perm[(h // 4) * 256 + (d // 32) * 128 + (h % 4) * 32 + (d % 32)] = h * D + d

    def kmaj(w, cols, kt, dtype):
        return np.ascontiguousarray(w.reshape(kt, 128, cols).transpose(1, 0, 2)).astype(dtype)

    shared = dict(
        wq_d=kmaj(wq_f[:, perm], C, 4, E4),
        wk_d=kmaj(wkv_f[:, :C][:, perm], C, 4, E4),
        wv_d=kmaj(wkv_f[:, C:], C, 4, E4),
        wp_d=np.ascontiguousarray(wp.reshape(H, D, C).transpose(1, 0, 2)).astype(E4),
        w1_d=kmaj(w1_f, HID, 4, BF),
        w2_d=np.ascontiguousarray(w2.reshape(16, 128, C).transpose(1, 0, 2)).astype(BF),
        bq_d=np.ascontiguousarray(bq_f[perm].reshape(4, 128).T).astype(f32),
        bk_d=np.ascontiguousarray(bkv_f[:C][perm].reshape(4, 128).T).astype(f32),
        bv_d=np.ascontiguousarray(bkv_f[C:]).astype(f32),
        bp_d=np.ascontiguousarray(bp).astype(f32),
        b1_d=np.ascontiguousarray(b1_f.reshape(16, 128).T).astype(f32),
        b2_d=np.ascontiguousarray(b2).astype(f32),
    )

    in_maps = []
    for core in range(NCORES):
        bi, qi = divmod(core, 4)
        in_maps.append(dict(shared,
                            xfull=np.ascontiguousarray(x[bi]),
                            xq=np.ascontiguousarray(x[bi, qi * NQ:(qi + 1) * NQ])))
    return in_maps


def _make_runner(nc):
    """Persistent jitted SPMD executor for `nc` (mirrors bass2jax.run_bass_via_pjrt
    but keeps the jitted function + avoids per-call retrace)."""
    import jax
    from jax.sharding import Mesh, PartitionSpec
    from jax.experimental.shard_map import shard_map
    import concourse.mybir as mybir
    from concourse import bass2jax

    bass2jax.install_neuronx_cc_hook()

    partition_name = nc.partition_id_tensor.name if nc.partition_id_tensor else None
    in_names, out_names, out_avals = [], [], []
    for alloc in nc.m.functions[0].allocations:
        if not isinstance(alloc, mybir.MemoryLocationSet):
            continue
        name = alloc.memorylocations[0].name
        if alloc.kind == "ExternalInput":
            if name != partition_name:
                in_names.append(name)
        elif alloc.kind == "ExternalOutput":
            out_names.append(name)
            out_avals.append(jax.core.ShapedArray(tuple(alloc.tensor_shape),
                                                  mybir.dt.np(alloc.dtype)))
    n_params = len(in_names)
    all_names = in_names + out_names
    if partition_name is not None:
        all_names = all_names + [partition_name]

    def _body(*args):
        operands = list(args)
        if partition_name is not None:
            operands.append(bass2jax.partition_id_tensor())
        outs = bass2jax._bass_exec_p.bind(
            *operands,
            out_avals=tuple(out_avals),
            in_names=tuple(all_names),
            out_names=tuple(out_names),
            lowering_input_output_aliases=(),
            sim_require_finite=True,
            sim_require_nnan=True,
            nc=nc,
        )
        return tuple(outs)

    devices = jax.devices()[:NCORES]
    mesh = Mesh(np.asarray(devices), ("core",))
    n_outs = len(out_names)
    sharded = jax.jit(
        shard_map(_body, mesh=mesh,
                  in_specs=(PartitionSpec("core"),) * (n_params + n_outs),
                  out_specs=(PartitionSpec("core"),) * n_outs,
                  check_rep=False),
        keep_unused=True,
    )

    def run(in_maps):
        concat_in = [np.concatenate([np.asarray(in_maps[c][name]) for c in range(NCORES)], axis=0)
                     for name in in_names]
        zeros = [np.zeros((NCORES * a.shape[0], *a.shape[1:]), a.dtype) for a in out_avals]
        out_arrs = sharded(*concat_in, *zeros)
        return [{name: np.asarray(out_arrs[i]).reshape(NCORES, *out_avals[i].shape)[c]
                 for i, name in enumerate(out_names)}
                for c in range(NCORES)]

    run.sharded = sharded
    run.in_names = in_names
    run.out_names = out_names
    run.out_avals = out_avals
    return run


def get_runner(repeat=1):
    key = f"runner{repeat}"
    if key not in _CACHE:
        _CACHE[key] = _make_runner(_build_program(repeat=repeat))
    return _CACHE[key]


def kernel(**inputs):
    runner = get_runner()
    in_maps = _prepare_host(inputs)
    results = runner(in_maps)
    out = np.empty((B, N, C), np.float32)
    for core in range(NCORES):
        bi, qi = divmod(core, 4)
        out[bi, qi * NQ:(qi + 1) * NQ] = results[core]["y"]
    return out


# revision 8
# speedup vs baseline: 1.4759x; 1.4759x over previous
"""Trainium2 Bass kernel for a pre-norm transformer block (attention + MLP).

Sharding: batch (2) x query-block (4) across 8 cores. Each core computes
LN1 + K/V over its full batch (replicated within its 4-core group) and
attention / projection / MLP for its own 1024 query tokens. No collectives.

Device layouts (per core):
  xT  : LN1(x) feature-major [128c, 4ct, 4096t] bf16
  kT  : per head-pair hp [128 (2 heads x 64d), 4096m] bf16
  v   : token-major [128m, 32mt, 8h, 65] bf16 (65th col = ones -> softmax denom)
  S^T : [128m, 2h, 512n] PSUM (keys on partitions; head pair row-packed)
  A@V : out^T[65, n]: lhsT=[V|1] per head, accumulated over 32 m-tiles
"""

import numpy as np
import ml_dtypes

B, N, C = 2, 4096, 512
H, D = 8, 64
HID = 2048
NQ = 1024
NCORES = 8
EPS = 1e-5
BF = ml_dtypes.bfloat16

_CACHE = {}


def _build_program(repeat=1):
    from concourse import bacc
    import concourse.bass as bass
    import concourse.mybir as mybir
    from concourse.tile import TileContext

    dt = mybir.dt
    AF = mybir.ActivationFunctionType
    ALU = mybir.AluOpType

    nc = bacc.Bacc(None, target_bir_lowering=False)

    xfull = nc.dram_tensor("xfull", (N, C), dt.float32, kind="ExternalInput")
    xq = nc.dram_tensor("xq", (NQ, C), dt.float32, kind="ExternalInput")
    wq_d = nc.dram_tensor("wq_d", (128, 4, C), dt.bfloat16, kind="ExternalInput")
    wk_d = nc.dram_tensor("wk_d", (128, 4, C), dt.bfloat16, kind="ExternalInput")
    wv_d = nc.dram_tensor("wv_d", (128, 4, C), dt.bfloat16, kind="ExternalInput")
    wp_d = nc.dram_tensor("wp_d", (128, 4, C), dt.bfloat16, kind="ExternalInput")
    w1_d = nc.dram_tensor("w1_d", (128, 4, HID), dt.bfloat16, kind="ExternalInput")
    w2_d = nc.dram_tensor("w2_d", (128, 16, C), dt.bfloat16, kind="ExternalInput")
    bq_d = nc.dram_tensor("bq_d", (128, 4), dt.float32, kind="ExternalInput")
    bk_d = nc.dram_tensor("bk_d", (128, 4), dt.float32, kind="ExternalInput")
    bv_d = nc.dram_tensor("bv_d", (C,), dt.float32, kind="ExternalInput")
    bp_d = nc.dram_tensor("bp_d", (C,), dt.float32, kind="ExternalInput")
    b1_d = nc.dram_tensor("b1_d", (128, 16), dt.float32, kind="ExternalInput")
    b2_d = nc.dram_tensor("b2_d", (C,), dt.float32, kind="ExternalInput")
    y = nc.dram_tensor("y", (NQ, C), dt.float32, kind="ExternalOutput")

    xfull_t = xfull.rearrange("(i p) c -> p i c", p=128)
    xq_t = xq.rearrange("(i p) c -> p i c", p=128)
    y_t = y.rearrange("(i p) c -> p i c", p=128)

    import contextlib
    with TileContext(nc) as tc:
      with (tc.For_i(0, repeat, 1) if repeat > 1 else contextlib.nullcontext()):
        R = "r0_"
        with tc.tile_pool(name=R + "pers", bufs=1) as pers, \
             tc.tile_pool(name=R + "stat", bufs=4) as statp, \
             tc.tile_pool(name=R + "stream", bufs=3) as stream, \
             tc.tile_pool(name=R + "pB", bufs=1) as pB, \
             tc.tile_pool(name=R + "kq", bufs=2) as kqp, \
             tc.tile_pool(name=R + "ptp", bufs=3) as ptp, \
             tc.tile_pool(name=R + "pall", bufs=1, space="PSUM") as pall:

            eps_t = pers.tile([128, 1], dt.float32, name=R + "eps")
            nc.vector.memset(eps_t, EPS)
            xq_sb = pers.tile([128, 8, C], dt.float32, name=R + "xq_sb")
            nc.sync.dma_start(out=xq_sb, in_=xq_t[:])
            bq_sb = pers.tile([128, 4], dt.float32, name=R + "bq_sb")
            bk_sb = pers.tile([128, 4], dt.float32, name=R + "bk_sb")
            bv_sb = pers.tile([128, 8, 64], dt.bfloat16, name=R + "bv_sb")
            bp_sb = pers.tile([128, C], dt.float32, name=R + "bp_sb")
            b1_sb = pers.tile([128, 16], dt.float32, name=R + "b1_sb")
            b2_sb = pers.tile([128, C], dt.float32, name=R + "b2_sb")
            nc.sync.dma_start(out=bq_sb, in_=bq_d[:])
            nc.sync.dma_start(out=bk_sb, in_=bk_d[:])
            nc.sync.dma_start(out=b1_sb, in_=b1_d[:])
            nc.gpsimd.dma_start(out=bv_sb, in_=bass.AP(tensor=bv_d, offset=0, ap=[[0, 128], [1, C]]))
            nc.gpsimd.dma_start(out=bp_sb, in_=bass.AP(tensor=bp_d, offset=0, ap=[[0, 128], [1, C]]))
            nc.gpsimd.dma_start(out=b2_sb, in_=bass.AP(tensor=b2_d, offset=0, ap=[[0, 128], [1, C]]))

            # phase-B persistent tiles (attention)
            wp_sb = pB.tile([128, 4, C], dt.bfloat16, name=R + "wp_sb")
            nc.gpsimd.dma_start(out=wp_sb, in_=wp_d[:])
            v_sb = pB.tile([128, 32, H, 65], dt.bfloat16, name=R + "v_sb")
            nc.vector.memset(v_sb[:, :, :, 64:65], 1.0)
            # n-major attention out [t%128, nb, h, d] and its feature-major transpose
            ao_nm = pB.tile([128, 8, H, 64], dt.bfloat16, name=R + "ao_nm")
            aoT = pB.tile([128, 8, 4, 128], dt.bfloat16, name=R + "aoT")

            def ln_group(src_dram_or_sb, g, xT, tag, from_sbuf=False):
                """4 token-tiles: 1 load, per-tile LN stats+normalize, 1 batched transpose.

                xT layout: [128 ci, nblk, 4 co, 128 t]."""
                eng = nc.sync if g % 2 == 0 else nc.scalar
                if from_sbuf:
                    xt4 = src_dram_or_sb
                else:
                    xt4 = stream.tile([128, 4, C], dt.float32, tag="lnx", bufs=2, name=f"{R}{tag}x{g}")
                    eng.dma_start(out=xt4, in_=src_dram_or_sb)
                xn4 = stream.tile([128, 4, C], dt.bfloat16, tag="lnn", bufs=3, name=f"{R}{tag}n{g}")
                mv4 = statp.tile([128, 4, 2], dt.float32, tag="lnmv", name=f"{R}{tag}mv{g}")
                for j in range(4):
                    stats = statp.tile([128, 6], dt.float32, tag="lnst", name=f"{R}{tag}st{4*g+j}")
                    nc.vector.bn_stats(stats, xt4[:, j, :])
                    nc.vector.bn_aggr(mv4[:, j, :], stats)
                # rstd = exp(-0.5*ln(var+eps)): keeps Act on the single ln/exp table
                lnv = statp.tile([128, 4], dt.float32, tag="lnlv", name=f"{R}{tag}lv{g}")
                nc.scalar.activation(lnv, mv4[:, :, 1], AF.Ln, bias=eps_t)
                rstd4 = statp.tile([128, 4], dt.float32, tag="lnrs", name=f"{R}{tag}rs{g}")
                nc.scalar.activation(rstd4, lnv, AF.Exp, scale=-0.5)
                nmr4 = statp.tile([128, 4], dt.float32, tag="lnnm", name=f"{R}{tag}nm{g}")
                nc.vector.tensor_tensor(out=nmr4, in0=mv4[:, :, 0], in1=rstd4, op=ALU.mult)
                nc.vector.tensor_scalar(nmr4, nmr4, -1.0, None, ALU.mult)
                for j in range(4):
                    nc.vector.tensor_scalar(xn4[:, j, :], xt4[:, j, :],
                                            rstd4[:, j:j + 1], nmr4[:, j:j + 1],
                                            ALU.mult, ALU.add)
                eng.dma_start(out=xT[:, 4 * g:4 * g + 4, :, :], in_=xn4, transpose=True)

            def emit_kq(hp, kT, qT, wk_sb, wq_sb, xT, xqT):
                for nch in range(8):
                    ps_k = pall.tile([128, 512], dt.float32, bufs=2, tag="psmall",
                                     name=f"{R}ps_k{hp}_{nch}")
                    for kt in range(4):
                        nc.tensor.matmul(ps_k, wk_sb[:, kt, hp * 128:(hp + 1) * 128],
                                         xT[:, 4 * nch:4 * nch + 4, kt, :],
                                         start=(kt == 0), stop=(kt == 3))
                    nc.vector.tensor_scalar(kT[:, nch * 512:(nch + 1) * 512],
                                            ps_k, bk_sb[:, hp:hp + 1], None, ALU.add)
                for nch in range(2):
                    ps_q = pall.tile([128, 512], dt.float32, bufs=2, tag="psmall",
                                     name=f"{R}ps_q{hp}_{nch}")
                    for kt in range(4):
                        nc.tensor.matmul(ps_q, wq_sb[:, kt, hp * 128:(hp + 1) * 128],
                                         xqT[:, 4 * nch:4 * nch + 4, kt, :],
                                         start=(kt == 0), stop=(kt == 3))
                    nc.vector.tensor_scalar(qT[:, nch * 512:(nch + 1) * 512],
                                            ps_q, bq_sb[:, hp:hp + 1], None, ALU.add)

            def emit_attention(hp, kT, qT):
                for nch in range(2):
                    nsl = slice(nch * 512, (nch + 1) * 512)
                    po = [pall.tile([128, 4, 65], dt.float32, tag=f"po{h}",
                                    name=f"{R}po{hp}_{nch}_{h}") for h in range(2)]
                    pts = {}
                    for mt in range(33):
                        if mt < 32:
                            msl = slice(mt * 128, (mt + 1) * 128)
                            ps_s = pall.tile([128, 2, 512], dt.float32, bufs=2,
                                             tag="ps_s", name=f"{R}ps_s{hp}_{nch}_{mt}")
                            nc.tensor.matmul(ps_s[:, 0, :], kT[0:64, msl], qT[0:64, nsl],
                                             start=True, stop=True)
                            nc.tensor.matmul(ps_s[:, 1, :], kT[64:128, msl], qT[64:128, nsl],
                                             start=True, stop=True, tile_position=(64, 0))
                            pt = ptp.tile([128, 2, 512], dt.bfloat16, bufs=4, tag="pt",
                                          name=f"{R}pt{hp}_{nch}_{mt}")
                            nc.scalar.activation(pt, ps_s, AF.Exp, scale=float(D) ** -0.5)
                            pts[mt] = pt
                        if mt >= 1:
                            ptm = pts.pop(mt - 1)
                            for h in range(2):
                                for c4 in range(4):
                                    nc.tensor.matmul(
                                        po[h][:, c4, :],
                                        ptm[:, h, c4 * 128:(c4 + 1) * 128],
                                        v_sb[:, mt - 1, 2 * hp + h, :],
                                        start=(mt - 1 == 0 and c4 == 0),
                                        stop=(mt - 1 == 31 and c4 == 3),
                                        skip_group_check=True)
                    for h in range(2):
                        for c4 in range(4):
                            nb = nch * 4 + c4
                            rden = statp.tile([128, 1], dt.float32, bufs=4, tag="rden",
                                              name=f"{R}rden{hp}_{nch}_{h}_{c4}")
                            nc.vector.reciprocal(rden, po[h][:, c4, 64:65])
                            nc.vector.tensor_scalar(ao_nm[:, nb, 2 * hp + h, :],
                                                    po[h][:, c4, 0:64], rden,
                                                    None, ALU.mult)

            # ======== phase A: LN1 + QKV (xT scoped) ========
            kqt = {}
            with tc.tile_pool(name=R + "pA", bufs=1) as pA:
                wq_sb = pA.tile([128, 4, C], dt.bfloat16, name=R + "wq_sb")
                wk_sb = pA.tile([128, 4, C], dt.bfloat16, name=R + "wk_sb")
                wv_sb = pA.tile([128, 4, C], dt.bfloat16, name=R + "wv_sb")
                nc.gpsimd.dma_start(out=wq_sb, in_=wq_d[:])
                nc.gpsimd.dma_start(out=wk_sb, in_=wk_d[:])
                nc.gpsimd.dma_start(out=wv_sb, in_=wv_d[:])
                xT = pA.tile([128, 32, 4, 128], dt.bfloat16, name=R + "xT")
                xqT = pA.tile([128, 8, 4, 128], dt.bfloat16, name=R + "xqT")
                kT0 = kqp.tile([128, N], dt.bfloat16, tag="kT", name=f"{R}kT0")
                qT0 = kqp.tile([128, NQ], dt.bfloat16, tag="qT", name=f"{R}qT0")
                kqt[0] = (kT0, qT0)

                def emit_v(mt):
                    ps_v = pall.tile([128, C], dt.float32, bufs=2, tag="psmall",
                                     name=f"{R}ps_v{mt}")
                    for kt in range(4):
                        nc.tensor.matmul(ps_v, xT[:, mt, kt, :],
                                         wv_sb[:, kt, :], start=(kt == 0), stop=(kt == 3))
                    nc.vector.tensor_tensor(
                        out=v_sb[:, mt, :, 0:64],
                        in0=ps_v.rearrange("p (h d) -> p h d", h=H),
                        in1=bv_sb, op=ALU.add)

                def emit_k_chunk(hp, kT, nch):
                    ps_k = pall.tile([128, 512], dt.float32, bufs=2, tag="psmall",
                                     name=f"{R}ps_k{hp}_{nch}")
                    for kt in range(4):
                        nc.tensor.matmul(ps_k, wk_sb[:, kt, hp * 128:(hp + 1) * 128],
                                         xT[:, 4 * nch:4 * nch + 4, kt, :],
                                         start=(kt == 0), stop=(kt == 3))
                    nc.vector.tensor_scalar(kT[:, nch * 512:(nch + 1) * 512],
                                            ps_k, bk_sb[:, hp:hp + 1], None, ALU.add)

                xq4_t = xq.rearrange("(gr j p) c -> p gr j c", p=128, j=4)
                for g in range(2):
                    ln_group(xq4_t[:, g, :, :], g, xqT, "lq")
                for nch in range(2):
                    ps_q = pall.tile([128, 512], dt.float32, bufs=2, tag="psmall",
                                     name=f"{R}ps_q0_{nch}")
                    for kt in range(4):
                        nc.tensor.matmul(ps_q, wq_sb[:, kt, 0:128],
                                         xqT[:, 4 * nch:4 * nch + 4, kt, :],
                                         start=(kt == 0), stop=(kt == 3))
                    nc.vector.tensor_scalar(qT0[:, nch * 512:(nch + 1) * 512],
                                            ps_q, bq_sb[:, 0:1], None, ALU.add)
                xf4_t = xfull.rearrange("(gr j p) c -> p gr j c", p=128, j=4)
                for g in range(8):
                    ln_group(xf4_t[:, g, :, :], g, xT, "l1")
                    emit_k_chunk(0, kT0, g)
                    for mt in range(4 * g, 4 * g + 4):
                        emit_v(mt)
                for hp in range(1, 4):
                    kT = kqp.tile([128, N], dt.bfloat16, tag="kT", name=f"{R}kT{hp}")
                    qT = kqp.tile([128, NQ], dt.bfloat16, tag="qT", name=f"{R}qT{hp}")
                    kqt[hp] = (kT, qT)
                    emit_kq(hp, kT, qT, wk_sb, wq_sb, xT, xqT)
                    emit_attention(hp - 1, *kqt[hp - 1])

            # ======== phase B tail: last attention + wp ========
            emit_attention(3, *kqt[3])
            with tc.tile_pool(name=R + "pC", bufs=1) as pC:
                w1_sb = pC.tile([128, 4, HID], dt.bfloat16, name=R + "w1_sb")
                w2_sb = pC.tile([128, 16, C], dt.bfloat16, name=R + "w2_sb")
                nc.gpsimd.dma_start(out=w1_sb, in_=w1_d[:])
                nc.gpsimd.dma_start(out=w2_sb, in_=w2_d[:])
                for nb in range(8):
                    nc.sync.dma_start(out=aoT[:, nb, :, :], in_=ao_nm[:, nb, :, :],
                                      transpose=True)
                for ns in range(8):
                    ps_p = pall.tile([128, C], dt.float32, bufs=2, tag="psmall",
                                     name=f"{R}ps_p{ns}")
                    for blk in range(4):
                        nc.tensor.matmul(ps_p, aoT[:, ns, blk, :], wp_sb[:, blk, :],
                                         start=(blk == 0), stop=(blk == 3))
                    nc.vector.tensor_tensor(out=xq_sb[:, ns, :], in0=xq_sb[:, ns, :],
                                            in1=ps_p, op=ALU.add)
                    nc.vector.tensor_tensor(out=xq_sb[:, ns, :], in0=xq_sb[:, ns, :],
                                            in1=bp_sb, op=ALU.add)

                # ======== phase C: LN2 + MLP ========
                x2T = pC.tile([128, 8, 4, 128], dt.bfloat16, name=R + "x2T")
                for g in range(2):
                    ln_group(xq_sb.rearrange("p (gr j) c -> p gr j c", j=4)[:, g, :, :],
                             g, x2T, "l2", from_sbuf=True)
                for nch in range(2):
                    h_sb = pC.tile([128, 16, 512], dt.bfloat16, tag="h_sb",
                                   name=f"{R}h_sb{nch}")
                    for pt_i in range(16):
                        ps_h = pall.tile([128, 512], dt.float32, bufs=2, tag="psmall",
                                         name=f"{R}ps_h{pt_i}_{nch}")
                        for kt in range(4):
                            nc.tensor.matmul(ps_h, w1_sb[:, kt, pt_i * 128:(pt_i + 1) * 128],
                                             x2T[:, 4 * nch:4 * nch + 4, kt, :],
                                             start=(kt == 0), stop=(kt == 3))
                        nc.vector.tensor_scalar(h_sb[:, pt_i, :],
                                                ps_h, b1_sb[:, pt_i:pt_i + 1], 0.0,
                                                ALU.add, ALU.max)
                    for ns in range(4 * nch, 4 * nch + 4):
                        ps_m = pall.tile([128, C], dt.float32, bufs=2, tag="psmall",
                                         name=f"{R}ps_m{ns}")
                        qsl = slice((ns - 4 * nch) * 128, (ns - 4 * nch + 1) * 128)
                        for kt in range(16):
                            nc.tensor.matmul(ps_m, h_sb[:, kt, qsl], w2_sb[:, kt, :],
                                             start=(kt == 0), stop=(kt == 15))
                        ot = stream.tile([128, C], dt.float32, tag="out", name=f"{R}out{ns}")
                        nc.vector.tensor_tensor(out=ot, in0=ps_m, in1=xq_sb[:, ns, :], op=ALU.add)
                        nc.vector.tensor_tensor(out=ot, in0=ot, in1=b2_sb, op=ALU.add)
                        nc.sync.dma_start(out=y_t[:, ns, :], in_=ot)

    # Restrict the act-table placement pass to the single table holding both
    # Ln and Exp (others emptied, not removed, to keep act_func_set_id stable).
    import concourse.bacc as bacc_mod
    orig_tables = bacc_mod.get_activation_tables
    keep = "natural_log_exp_and_others"

    def _only_nl_exp(arch):
        return {k: (v if k == keep else set())
                for k, v in orig_tables(arch).items()}

    bacc_mod.get_activation_tables = _only_nl_exp
    try:
        nc.finalize()
    finally:
        bacc_mod.get_activation_tables = orig_tables
    return nc


def _prepare_host(inputs):
    f32 = np.float32
    x = np.asarray(inputs["x"], f32)
    ln1_w = np.asarray(inputs["ln1_w"], f32); ln1_b = np.asarray(inputs["ln1_b"], f32)
    ln2_w = np.asarray(inputs["ln2_w"], f32); ln2_b = np.asarray(inputs["ln2_b"], f32)
    wq = np.asarray(inputs["wq"], f32); wkv = np.asarray(inputs["wkv"], f32)
    wp = np.asarray(inputs["wp"], f32); bp = np.asarray(inputs["bp"], f32)
    w1 = np.asarray(inputs["w1"], f32); b1 = np.asarray(inputs["b1"], f32)
    w2 = np.asarray(inputs["w2"], f32); b2 = np.asarray(inputs["b2"], f32)

    wq_f = ln1_w[:, None] * wq
    wkv_f = ln1_w[:, None] * wkv
    w1_f = ln2_w[:, None] * w1
    bq_f = ln1_b @ wq
    bkv_f = ln1_b @ wkv
    b1_f = b1 + ln2_b @ w1

    def kmaj(w, cols, kt):
        return np.ascontiguousarray(w.reshape(kt, 128, cols).transpose(1, 0, 2)).astype(BF)

    shared = dict(
        wq_d=kmaj(wq_f, C, 4),
        wk_d=kmaj(wkv_f[:, :C], C, 4),
        wv_d=kmaj(wkv_f[:, C:], C, 4),
        wp_d=kmaj(wp, C, 4),
        w1_d=kmaj(w1_f, HID, 4),
        w2_d=np.ascontiguousarray(w2.reshape(16, 128, C).transpose(1, 0, 2)).astype(BF),
        bq_d=np.ascontiguousarray(bq_f.reshape(4, 128).T).astype(f32),
        bk_d=np.ascontiguousarray(bkv_f[:C].reshape(4, 128).T).astype(f32),
        bv_d=np.ascontiguousarray(bkv_f[C:]).astype(f32),
        bp_d=np.ascontiguousarray(bp).astype(f32),
        b1_d=np.ascontiguousarray(b1_f.reshape(16, 128).T).astype(f32),
        b2_d=np.ascontiguousarray(b2).astype(f32),
    )

    in_maps = []
    for core in range(NCORES):
        bi, qi = divmod(core, 4)
        in_maps.append(dict(shared,
                            xfull=np.ascontiguousarray(x[bi]),
                            xq=np.ascontiguousarray(x[bi, qi * NQ:(qi + 1) * NQ])))
    return in_maps


def _make_runner(nc):
    """Persistent jitted SPMD executor for `nc` (mirrors bass2jax.run_bass_via_pjrt
    but keeps the jitted function + avoids per-call retrace)."""
    import jax
    import numpy as jnp_np
    from jax.sharding import Mesh, PartitionSpec
    from jax.experimental.shard_map import shard_map
    import concourse.mybir as mybir
    from concourse import bass2jax

    bass2jax.install_neuronx_cc_hook()

    partition_name = nc.partition_id_tensor.name if nc.partition_id_tensor else None
    in_names, out_names, out_avals = [], [], []
    for alloc in nc.m.functions[0].allocations:
        if not isinstance(alloc, mybir.MemoryLocationSet):
            continue
        name = alloc.memorylocations[0].name
        if alloc.kind == "ExternalInput":
            if name != partition_name:
                in_names.append(name)
        elif alloc.kind == "ExternalOutput":
            out_names.append(name)
            out_avals.append(jax.core.ShapedArray(tuple(alloc.tensor_shape),
                                                  mybir.dt.np(alloc.dtype)))
    n_params = len(in_names)
    all_names = in_names + out_names
    if partition_name is not None:
        all_names = all_names + [partition_name]

    def _body(*args):
        operands = list(args)
        if partition_name is not None:
            operands.append(bass2jax.partition_id_tensor())
        outs = bass2jax._bass_exec_p.bind(
            *operands,
            out_avals=tuple(out_avals),
            in_names=tuple(all_names),
            out_names=tuple(out_names),
            lowering_input_output_aliases=(),
            sim_require_finite=True,
            sim_require_nnan=True,
            nc=nc,
        )
        return tuple(outs)

    devices = jax.devices()[:NCORES]
    mesh = Mesh(np.asarray(devices), ("core",))
    n_outs = len(out_names)
    sharded = jax.jit(
        shard_map(_body, mesh=mesh,
                  in_specs=(PartitionSpec("core"),) * (n_params + n_outs),
                  out_specs=(PartitionSpec("core"),) * n_outs,
                  check_rep=False),
        keep_unused=True,
    )

    def run(in_maps):
        concat_in = [np.concatenate([np.asarray(in_maps[c][name]) for c in range(NCORES)], axis=0)
                     for name in in_names]
        zeros = [np.zeros((NCORES * a.shape[0], *a.shape[1:]), a.dtype) for a in out_avals]
        out_arrs = sharded(*concat_in, *zeros)
        return [{name: np.asarray(out_arrs[i]).reshape(NCORES, *out_avals[i].shape)[c]
                 for i, name in enumerate(out_names)}
                for c in range(NCORES)]

    run.sharded = sharded
    run.in_names = in_names
    run.out_names = out_names
    run.out_avals = out_avals
    return run


def get_runner(repeat=1):
    key = f"runner{repeat}"
    if key not in _CACHE:
        _CACHE[key] = _make_runner(_build_program(repeat=repeat))
    return _CACHE[key]


def kernel(**inputs):
    runner = get_runner()
    in_maps = _prepare_host(inputs)
    results = runner(in_maps)
    out = np.empty((B, N, C), np.float32)
    for core in range(NCORES):
        bi, qi = divmod(core, 4)
        out[bi, qi * NQ:(qi + 1) * NQ] = results[core]["y"]
    return out

